# revision 13
# baseline (speedup 1.0000x reference)
"""Trainium2 Bass kernel for nn_ButterflyFactorNewMlp.

Computes: attn = einsum('ds,td->st', w1, w2) * sparse_mask
          out  = gelu(einsum('bds,st->bdt', x, attn) + b2)   (exact erf gelu)

Key structural fact: sparse_mask[s,t] != 0 iff s//81 == t//81 and
(s%27)//3 == (t%27)//3.  Grouping features by g = (s//81, (s%27)//3)
(81 groups of 9) makes attn block-diagonal with 81 independent 9x9
blocks: out[:, group g] depends ONLY on x[:, group g].

Sharding: output-block parallel, fully balanced.  Each core owns 10 of
the 81 blocks (90 feature columns) for ALL 49152 tokens, and the 81st
block is shared: every core computes it for its own 1/8 slice of the
tokens as a separate tiny matmul stream.  Per-core DMA is exactly the
balanced floor: x 8.85+0.11 MB in, out 8.85+0.11 MB back, plus only the
w1/w2 columns of its own blocks (1.2 MB vs 8.6 MB replicated).  No
collectives (any on-device collective costs ~100us here).

The per-core wall clock is DMA-bus-bound (~360 GB/s spec, ~270-310
practical per core with all 8 streaming), so the kernel is organized
around keeping that bus busy with zero waste:
  - all DMAs sized so the outer dim divides by 16/15 (queue spray rule:
    fan-out = largest divisor of the outer dim <= 16, ~25 GB/s/queue)
  - x loads ride the Sync-engine HWDGE ring alone, in issue order, with
    a 3-deep SBUF ring as throttle (concurrent DMAs fair-share the bus,
    so unbounded prefetch makes every piece arrive late)
  - output stores ride the Activation-engine HWDGE ring, extra-block
    traffic rides the software-DGE ring
  - first token pieces are small so the gelu chain starts early

Device program per core (identical NEFF on all 8):
  stage 1: attn[99,99] = sum over 23 d-chunks of w1cT @ w2c (PE, fp16),
           masked by a 0/1 window (DVE) -> SBUF fp16.  Rows/cols 0:90
           are the core's own blocks, 90:99 the shared block.
  extra:   the shared block's [9,9] corner is copied to partitions 0:9
           (SWDGE sbuf->sbuf) and 12 N=512 matmuls + 3 [9,2048]
           activations produce out_extra[9,6144] during the window when
           ScalarE would otherwise idle waiting for x.
  stage 2: per 512-token slice: ps[90,512] = matmul(lhsT=attn[0:90,
           0:90], rhs=xT[0:90, 512]); 4 slices fill a 4-bank psum group
           and one ACTIVATE applies per-partition bias + exact-erf gelu
           into fp16; two 4-bank groups ping-pong.

Precision: fp16 inputs/weights, fp32 PSUM accumulation, gelu on the
fp32 accumulator, fp16 stores -> end-to-end ~7e-4 relative error.
"""

import sys

if "/opt/trn_rl_repo" not in sys.path:
    sys.path.insert(0, "/opt/trn_rl_repo")

import numpy as np

import concourse.bacc as bacc
import concourse.mybir as mybir
import concourse.tile as tile
from concourse.bass_utils import run_bass_kernel_spmd

F32 = mybir.dt.float32
F16 = mybir.dt.float16
GELU = mybir.ActivationFunctionType.Gelu

N_CORES = 8
B, D, S = 64, 768, 729
H = 2916
HP = 2944                      # hidden padded to 23*128
N_KD = HP // 128               # 23 contraction chunks for stage 1
M_ALL = B * D                  # 49152 tokens, all processed by every core
MAIN = 90                      # own feature columns: 10 blocks * 9
XB = 9                         # shared-block width
TP = MAIN + XB                 # stage-1 window width
M_X = M_ALL // N_CORES         # 6144 shared-block tokens per core
# token pieces: small pieces at both ends — the first so the act chain
# starts early, the last so the final store flush is short
PIECES = [2048, 2048, 4096, 8192, 8192, 8192, 8192, 4096, 2048, 2048]
GRP = 2048                     # tokens per activation group (4 psum banks)
MM_N = 512                     # tokens per matmul (1 psum bank, fp32)

_COMPILED = None
LAST = None  # BassKernelResults of the most recent kernel() call (for test.py)


def _build():
    nc = bacc.Bacc("TRN2", target_bir_lowering=False, debug=False)

    x_d = nc.dram_tensor("xT", [MAIN, M_ALL], F16, kind="ExternalInput")
    xx_d = nc.dram_tensor("xX", [XB, M_X], F16, kind="ExternalInput")
    w1_d = nc.dram_tensor("w1p", [128, N_KD, TP], F16, kind="ExternalInput")
    w2_d = nc.dram_tensor("w2p", [128, N_KD, TP], F16, kind="ExternalInput")
    mw_d = nc.dram_tensor("maskw", [TP, TP], F16, kind="ExternalInput")
    b2_d = nc.dram_tensor("b2p", [MAIN, 1], F32, kind="ExternalInput")
    b2x_d = nc.dram_tensor("b2x", [XB, 1], F32, kind="ExternalInput")
    out_d = nc.dram_tensor("out", [MAIN, M_ALL], F16, kind="ExternalOutput")
    outx_d = nc.dram_tensor("outx", [XB, M_X], F16, kind="ExternalOutput")

    xoff = [0]
    for psz in PIECES:
        xoff.append(xoff[-1] + psz)

    with tile.TileContext(nc) as tc:
        with (
            tc.tile_pool(name="const", bufs=1) as cpool,
            tc.tile_pool(name="xin", bufs=4) as xpool,
            tc.tile_pool(name="oout", bufs=3) as opool,
            tc.tile_pool(name="ps", bufs=2, space="PSUM") as pspool,
        ):
            # ---- const loads: only w and the first x piece sit in the
            # critical prefix; everything else follows ----
            KH = 12
            w1_sb = cpool.tile([128, N_KD, TP], F16)
            w2_sb = cpool.tile([128, N_KD, TP], F16)
            nc.sync.dma_start(w1_sb[:, 0:KH, :], w1_d[:, 0:KH, :])
            nc.sync.dma_start(w2_sb[:, 0:KH, :], w2_d[:, 0:KH, :])
            nc.sync.dma_start(w1_sb[:, KH:N_KD, :], w1_d[:, KH:N_KD, :])
            nc.sync.dma_start(w2_sb[:, KH:N_KD, :], w2_d[:, KH:N_KD, :])

            # x prefetch, throttled by the pool ring: concurrent DMAs
            # fair-share the queues, so in-flight depth must stay small for
            # pieces to complete in issue order
            x_sbs = {}

            def fetch_x(p):
                if p >= len(PIECES):
                    return
                x_sb = xpool.tile([MAIN, PIECES[p]], F16, tag="x", name=f"x{p}",
                                  padded_shape=[MAIN, max(PIECES)])
                nc.sync.dma_start(x_sb[:], x_d[:, xoff[p] : xoff[p + 1]])
                x_sbs[p] = x_sb

            for p in range(2):
                fetch_x(p)

            # small consts ride the software-DGE path
            mw_sb = cpool.tile([TP, TP], F16)
            nc.gpsimd.dma_start(mw_sb[:], mw_d[:])
            b2_sb = cpool.tile([MAIN, 1], F32)
            nc.gpsimd.dma_start(b2_sb[:], b2_d[:])
            b2x_sb = cpool.tile([XB, 1], F32)
            nc.gpsimd.dma_start(b2x_sb[:], b2x_d[:])
            # shared-block x: needed only at the tail, software-DGE is fine
            xx_sb = cpool.tile([XB, M_X], F16)
            nc.gpsimd.dma_start(xx_sb[:], xx_d[:])

            # warm the gelu LUT during the DMA shadow
            warm = cpool.tile([1, 1], F32)
            nc.gpsimd.memset(warm[:], 0.0)
            nc.scalar.activation(warm[:], warm[:], GELU)

            # ---- stage 1: this core's diagonal attn window ----
            ps1 = pspool.tile([TP, GRP], F32, tag="ps", name="ps1")
            for kd in range(N_KD):
                nc.tensor.matmul(
                    ps1[:, 0:TP],
                    w1_sb[:, kd, :],
                    w2_sb[:, kd, :],
                    start=(kd == 0),
                    stop=(kd == N_KD - 1),
                )
            attn_sb = cpool.tile([TP, TP], F16)
            nc.vector.tensor_tensor(
                attn_sb[:], ps1[:, 0:TP], mw_sb[:], mybir.AluOpType.mult
            )
            # shared block's 9x9 corner moved to partitions 0:9 (sbuf->sbuf)
            attn_x = cpool.tile([XB, XB], F16)
            nc.gpsimd.dma_start(attn_x[:], attn_sb[MAIN:TP, MAIN:TP])

            # ---- stage 2: all tokens through the core's own 10 blocks ----
            off = 0
            for p, psz in enumerate(PIECES):
                fetch_x(p + 2)
                x_sb = x_sbs[p]
                o_sb = opool.tile([MAIN, psz], F16, tag="o", name="o_sb",
                                  padded_shape=[MAIN, max(PIECES)])
                for g in range(psz // GRP):
                    ps = pspool.tile([MAIN, GRP], F32, tag="ps", name="ps")
                    for s in range(GRP // MM_N):
                        nc.tensor.matmul(
                            ps[:, s * MM_N : (s + 1) * MM_N],
                            attn_sb[0:MAIN, 0:MAIN],
                            x_sb[:, g * GRP + s * MM_N : g * GRP + (s + 1) * MM_N],
                            start=True,
                            stop=True,
                        )
                    nc.scalar.activation(
                        o_sb[:, g * GRP : (g + 1) * GRP], ps[:], GELU, bias=b2_sb[:]
                    )
                # store issued on the sync ring right after the next fetch,
                # so fetches are never queued behind store waits
                nc.sync.dma_start(out_d[:, off : off + psz], o_sb[:])
                off += psz

            # ---- shared block: 6144 tokens through a [9,9] stationary,
            # rides the pipeline tail while the last stores flush ----
            for xg in range(M_X // GRP):
                psx = pspool.tile([XB, GRP], F32, tag="ps", name="psx")
                for s in range(GRP // MM_N):
                    t0 = xg * GRP + s * MM_N
                    nc.tensor.matmul(
                        psx[:, s * MM_N : (s + 1) * MM_N],
                        attn_x[:],
                        xx_sb[:, t0 : t0 + MM_N],
                        start=True,
                        stop=True,
                    )
                ox_sb = opool.tile([XB, GRP], F16, tag="ox", name="ox_sb", bufs=2)
                nc.scalar.activation(ox_sb[:], psx[:], GELU, bias=b2x_sb[:])
                nc.gpsimd.dma_start(
                    outx_d[:, xg * GRP : (xg + 1) * GRP], ox_sb[:]
                )

    nc.compile()
    return nc


def _group_perm():
    """Feature order grouping s by (s//81, (s%27)//3): 81 groups of 9."""
    p = []
    for blk in range(9):
        for bb in range(9):
            for a in range(3):
                for c in range(3):
                    p.append(81 * blk + 27 * a + 3 * bb + c)
    return np.asarray(p)


def _pack_w(wcols):
    """[H, TP] f32 -> partition-major [128, N_KD, TP] fp16 (zero padded)."""
    wpad = np.zeros((HP, TP), np.float32)
    wpad[:H] = wcols
    return np.ascontiguousarray(
        wpad.reshape(N_KD, 128, TP).transpose(1, 0, 2)
    ).astype(np.float16)


def kernel(x, w1, w2, b2, sparse_mask):
    global _COMPILED, LAST
    if _COMPILED is None:
        _COMPILED = _build()
    nc = _COMPILED

    x = np.asarray(x, dtype=np.float32)
    w1 = np.asarray(w1, dtype=np.float32)
    w2 = np.asarray(w2, dtype=np.float32)
    b2 = np.asarray(b2, dtype=np.float32)
    mask = np.asarray(sparse_mask, dtype=np.float32)

    perm = _group_perm()
    xcols = perm[MAIN * N_CORES :]          # shared block, all cores
    xf = x.reshape(M_ALL, S)

    in_maps = []
    for c in range(N_CORES):
        mcols = perm[MAIN * c : MAIN * (c + 1)]   # own 10 blocks
        cols = np.concatenate([mcols, xcols])     # stage-1 window order

        in_maps.append(
            {
                "xT": np.ascontiguousarray(xf[:, mcols].T, dtype=np.float16),
                "xX": np.ascontiguousarray(
                    xf[c * M_X : (c + 1) * M_X, xcols].T, dtype=np.float16
                ),
                "w1p": _pack_w(w1[:, cols]),
                "w2p": _pack_w(w2[cols, :].T),
                "maskw": mask[np.ix_(cols, cols)].astype(np.float16),
                "b2p": np.ascontiguousarray(
                    b2[mcols].reshape(MAIN, 1), dtype=np.float32
                ),
                "b2x": np.ascontiguousarray(
                    b2[xcols].reshape(XB, 1), dtype=np.float32
                ),
            }
        )

    LAST = run_bass_kernel_spmd(nc, in_maps, list(range(N_CORES)))

    out = np.empty((M_ALL, S), np.float32)
    for c in range(N_CORES):
        mcols = perm[MAIN * c : MAIN * (c + 1)]
        out[:, mcols] = LAST.results[c]["out"].T.astype(np.float32)
        out[c * M_X : (c + 1) * M_X, xcols] = (
            LAST.results[c]["outx"].T.astype(np.float32)
        )
    return out.reshape(B, D, S)


# revision 21
# speedup vs baseline: 1.0795x; 1.0795x over previous
"""Trainium2 Bass kernel for nn_ButterflyFactorNewMlp.

Computes: attn = einsum('ds,td->st', w1, w2) * sparse_mask
          out  = gelu(einsum('bds,st->bdt', x, attn) + b2)   (exact erf gelu)

Key structural fact: sparse_mask[s,t] != 0 iff s//81 == t//81 and
(s%27)//3 == (t%27)//3.  Grouping features by g = (s//81, (s%27)//3)
(81 groups of 9) makes attn block-diagonal with 81 independent 9x9
blocks: out[:, group g] depends ONLY on x[:, group g].

Sharding: output-block parallel, fully balanced.  Each core owns 10 of
the 81 blocks (90 feature columns) for ALL 49152 tokens, and the 81st
block is shared: every core computes it for its own 1/8 slice of the
tokens as a separate tiny matmul stream.  Per-core DMA is exactly the
balanced floor: x 8.85+0.11 MB in, out 8.85+0.11 MB back, plus only the
w1/w2 columns of its own blocks (1.2 MB vs 8.6 MB replicated).  No
collectives (any on-device collective costs ~100us here).

The per-core wall clock is DMA-bus-bound (~360 GB/s spec, ~270-310
practical per core with all 8 streaming), so the kernel is organized
around keeping that bus busy with zero waste:
  - all DMAs sized so the outer dim divides by 16/15 (queue spray rule:
    fan-out = largest divisor of the outer dim <= 16, ~25 GB/s/queue)
  - x loads ride the Sync-engine HWDGE ring alone, in issue order, with
    a 3-deep SBUF ring as throttle (concurrent DMAs fair-share the bus,
    so unbounded prefetch makes every piece arrive late)
  - output stores ride the Activation-engine HWDGE ring, extra-block
    traffic rides the software-DGE ring
  - first token pieces are small so the gelu chain starts early

Device program per core (identical NEFF on all 8):
  stage 1: attn[99,99] = sum over 23 d-chunks of w1cT @ w2c (PE, fp16),
           masked by a 0/1 window (DVE) -> SBUF fp16.  Rows/cols 0:90
           are the core's own blocks, 90:99 the shared block.
  extra:   the shared block's [9,9] corner is copied to partitions 0:9
           (SWDGE sbuf->sbuf) and 12 N=512 matmuls + 3 [9,2048]
           activations produce out_extra[9,6144] during the window when
           ScalarE would otherwise idle waiting for x.
  stage 2: per 512-token slice: ps[90,512] = matmul(lhsT=attn[0:90,
           0:90], rhs=xT[0:90, 512]); 4 slices fill a 4-bank psum group
           and one ACTIVATE applies per-partition bias + exact-erf gelu
           into fp16; two 4-bank groups ping-pong.

Precision: fp16 inputs/weights, fp32 PSUM accumulation, gelu on the
fp32 accumulator, fp16 stores -> end-to-end ~7e-4 relative error.
"""

import sys

if "/opt/trn_rl_repo" not in sys.path:
    sys.path.insert(0, "/opt/trn_rl_repo")

import numpy as np

import concourse.bacc as bacc
import concourse.mybir as mybir
import concourse.tile as tile
from concourse.bass_utils import run_bass_kernel_spmd

F32 = mybir.dt.float32
F16 = mybir.dt.float16
GELU = mybir.ActivationFunctionType.Gelu

N_CORES = 8
B, D, S = 64, 768, 729
H = 2916
HP = 2944                      # hidden padded to 23*128
N_KD = HP // 128               # 23 contraction chunks for stage 1
M_ALL = B * D                  # 49152 tokens, all processed by every core
MAIN = 90                      # own feature columns: 10 blocks * 9
XB = 9                         # shared-block width
TP = MAIN + XB                 # stage-1 window width
M_X = M_ALL // N_CORES         # 6144 shared-block tokens per core
# token pieces: small pieces at both ends — the first so the act chain
# starts early, the last so the final store flush is short; the tapered
# tail pieces share one store
PIECES = [2048, 2048, 4096, 8192, 8192, 8192, 8192, 4096, 2048, 2048]
STORE_GROUPS = [1, 1, 1, 1, 1, 1, 1, 3]   # pieces per output store
GRP = 2048                     # tokens per activation group (4 psum banks)
MM_N = 512                     # tokens per matmul (1 psum bank, fp32)

_COMPILED = None
LAST = None  # BassKernelResults of the most recent kernel() call (for test.py)


def _build():
    nc = bacc.Bacc("TRN2", target_bir_lowering=False, debug=False)

    x_d = nc.dram_tensor("xT", [MAIN, M_ALL], F16, kind="ExternalInput")
    xx_d = nc.dram_tensor("xX", [XB, M_X], F16, kind="ExternalInput")
    w1_d = nc.dram_tensor("w1p", [128, N_KD, TP], F16, kind="ExternalInput")
    w2_d = nc.dram_tensor("w2p", [128, N_KD, TP], F16, kind="ExternalInput")
    mw_d = nc.dram_tensor("maskw", [TP, TP], F16, kind="ExternalInput")
    b2_d = nc.dram_tensor("b2p", [MAIN, 1], F32, kind="ExternalInput")
    b2x_d = nc.dram_tensor("b2x", [128, 1], F32, kind="ExternalInput")
    out_d = nc.dram_tensor("out", [MAIN, M_ALL], F16, kind="ExternalOutput")
    outx_d = nc.dram_tensor("outx", [XB, M_X], F16, kind="ExternalOutput")

    xoff = [0]
    for psz in PIECES:
        xoff.append(xoff[-1] + psz)

    with tile.TileContext(nc) as tc:
        with (
            tc.tile_pool(name="const", bufs=1) as cpool,
            tc.tile_pool(name="xin", bufs=3) as xpool,
            tc.tile_pool(name="oout", bufs=3) as opool,
            tc.tile_pool(name="ps", bufs=2, space="PSUM") as pspool,
        ):
            # ---- const loads: only w and the first x piece sit in the
            # critical prefix; everything else follows ----
            KH = 12
            w1_sb = cpool.tile([128, N_KD, TP], F16)
            w2_sb = cpool.tile([128, N_KD, TP], F16)
            nc.sync.dma_start(w1_sb[:, 0:KH, :], w1_d[:, 0:KH, :])
            nc.sync.dma_start(w2_sb[:, 0:KH, :], w2_d[:, 0:KH, :])
            nc.sync.dma_start(w1_sb[:, KH:N_KD, :], w1_d[:, KH:N_KD, :])
            nc.sync.dma_start(w2_sb[:, KH:N_KD, :], w2_d[:, KH:N_KD, :])

            # x prefetch, throttled by the pool ring: concurrent DMAs
            # fair-share the queues, so in-flight depth must stay small for
            # pieces to complete in issue order
            x_sbs = {}

            def fetch_x(p):
                if p >= len(PIECES):
                    return
                x_sb = xpool.tile([MAIN, PIECES[p]], F16, tag="x", name=f"x{p}",
                                  padded_shape=[MAIN, max(PIECES)])
                nc.sync.dma_start(x_sb[:], x_d[:, xoff[p] : xoff[p + 1]])
                x_sbs[p] = x_sb

            for p in range(2):
                fetch_x(p)

            # small consts ride the software-DGE path
            mw_sb = cpool.tile([TP, TP], F16)
            nc.gpsimd.dma_start(mw_sb[:], mw_d[:])
            b2_sb = cpool.tile([MAIN, 1], F32)
            nc.gpsimd.dma_start(b2_sb[:], b2_d[:])
            b2x_sb = cpool.tile([128, 1], F32)
            nc.gpsimd.dma_start(b2x_sb[:], b2x_d[:])
            # shared-block x: needed only at the tail, software-DGE is fine
            xx_sb = cpool.tile([XB, M_X], F16)
            nc.gpsimd.dma_start(xx_sb[:], xx_d[:])

            # warm the gelu LUT during the DMA shadow
            warm = cpool.tile([1, 1], F32)
            nc.gpsimd.memset(warm[:], 0.0)
            nc.scalar.activation(warm[:], warm[:], GELU)

            # ---- stage 1: this core's diagonal attn window ----
            ps1 = pspool.tile([TP, GRP], F32, tag="ps", name="ps1")
            for kd in range(N_KD):
                nc.tensor.matmul(
                    ps1[:, 0:TP],
                    w1_sb[:, kd, :],
                    w2_sb[:, kd, :],
                    start=(kd == 0),
                    stop=(kd == N_KD - 1),
                )
            attn_sb = cpool.tile([TP, TP], F16)
            nc.vector.tensor_tensor(
                attn_sb[:], ps1[:, 0:TP], mw_sb[:], mybir.AluOpType.mult
            )
            # shared block's 9x9 corner moved to partitions 0:9 (sbuf->sbuf)
            attn_x = cpool.tile([XB, XB], F16)
            nc.gpsimd.dma_start(attn_x[:], attn_sb[MAIN:TP, MAIN:TP])

            # ---- stage 2: all tokens through the core's own 10 blocks ----
            # pieces grouped per output store; stores ride the Activation
            # ring so the sync ring stays a pure in-order fetch stream
            p = 0
            off = 0
            for grp_pieces in STORE_GROUPS:
                gsz = sum(PIECES[p + i] for i in range(grp_pieces))
                o_sb = opool.tile([MAIN, gsz], F16, tag="o", name="o_sb",
                                  padded_shape=[MAIN, max(PIECES)])
                o_off = 0
                for _ in range(grp_pieces):
                    psz = PIECES[p]
                    fetch_x(p + 2)
                    x_sb = x_sbs[p]
                    for g in range(psz // GRP):
                        ps = pspool.tile([MAIN, GRP], F32, tag="ps", name="ps")
                        for s in range(GRP // MM_N):
                            nc.tensor.matmul(
                                ps[:, s * MM_N : (s + 1) * MM_N],
                                attn_sb[0:MAIN, 0:MAIN],
                                x_sb[:, g * GRP + s * MM_N
                                     : g * GRP + (s + 1) * MM_N],
                                start=True,
                                stop=True,
                            )
                        nc.scalar.activation(
                            o_sb[:, o_off + g * GRP : o_off + (g + 1) * GRP],
                            ps[:], GELU, bias=b2_sb[:],
                        )
                    o_off += psz
                    p += 1
                nc.scalar.dma_start(out_d[:, off : off + gsz], o_sb[:])
                off += gsz

            # ---- shared block: 6144 tokens through a [9,9] stationary,
            # stacked 4x1536 at partition offsets 0/32/64/96 so ONE
            # activation covers it; rides the tail under the store flush ----
            psx = pspool.tile([73, GRP], F32, tag="ps", name="psx")
            nc.vector.memset(psx[:], 0.0)
            for k in range(3):
                for s in range(GRP // MM_N):
                    t0 = k * GRP + s * MM_N
                    nc.tensor.matmul(
                        psx[32 * k : 32 * k + XB, s * MM_N : (s + 1) * MM_N],
                        attn_x[:],
                        xx_sb[:, t0 : t0 + MM_N],
                        start=True,
                        stop=True,
                    )
            ox_sb = opool.tile([73, GRP], F16, tag="ox", name="ox_sb",
                               bufs=1)
            nc.scalar.activation(ox_sb[:], psx[:], GELU, bias=b2x_sb[0:73])
            for k in range(3):
                nc.gpsimd.dma_start(
                    outx_d[:, k * GRP : (k + 1) * GRP],
                    ox_sb[32 * k : 32 * k + XB, :],
                )

    nc.compile()
    return nc


def _group_perm():
    """Feature order grouping s by (s//81, (s%27)//3): 81 groups of 9."""
    p = []
    for blk in range(9):
        for bb in range(9):
            for a in range(3):
                for c in range(3):
                    p.append(81 * blk + 27 * a + 3 * bb + c)
    return np.asarray(p)


def _pack_w(wcols):
    """[H, TP] f32 -> partition-major [128, N_KD, TP] fp16 (zero padded)."""
    wpad = np.zeros((HP, TP), np.float32)
    wpad[:H] = wcols
    return np.ascontiguousarray(
        wpad.reshape(N_KD, 128, TP).transpose(1, 0, 2)
    ).astype(np.float16)


def kernel(x, w1, w2, b2, sparse_mask):
    global _COMPILED, LAST
    if _COMPILED is None:
        _COMPILED = _build()
    nc = _COMPILED

    x = np.asarray(x, dtype=np.float32)
    w1 = np.asarray(w1, dtype=np.float32)
    w2 = np.asarray(w2, dtype=np.float32)
    b2 = np.asarray(b2, dtype=np.float32)
    mask = np.asarray(sparse_mask, dtype=np.float32)

    perm = _group_perm()
    xcols = perm[MAIN * N_CORES :]          # shared block, all cores
    xf = x.reshape(M_ALL, S)
    # shared-block bias replicated at the 4 partition stack offsets
    b2x_stack = np.zeros((128, 1), np.float32)
    for k in range(3):
        b2x_stack[32 * k : 32 * k + XB, 0] = b2[xcols]

    in_maps = []
    for c in range(N_CORES):
        mcols = perm[MAIN * c : MAIN * (c + 1)]   # own 10 blocks
        cols = np.concatenate([mcols, xcols])     # stage-1 window order

        in_maps.append(
            {
                "xT": np.ascontiguousarray(xf[:, mcols].T, dtype=np.float16),
                "xX": np.ascontiguousarray(
                    xf[c * M_X : (c + 1) * M_X, xcols].T, dtype=np.float16
                ),
                "w1p": _pack_w(w1[:, cols]),
                "w2p": _pack_w(w2[cols, :].T),
                "maskw": mask[np.ix_(cols, cols)].astype(np.float16),
                "b2p": np.ascontiguousarray(
                    b2[mcols].reshape(MAIN, 1), dtype=np.float32
                ),
                "b2x": b2x_stack,
            }
        )

    LAST = run_bass_kernel_spmd(nc, in_maps, list(range(N_CORES)))

    out = np.empty((M_ALL, S), np.float32)
    for c in range(N_CORES):
        mcols = perm[MAIN * c : MAIN * (c + 1)]
        out[:, mcols] = LAST.results[c]["out"].T.astype(np.float32)
        out[c * M_X : (c + 1) * M_X, xcols] = (
            LAST.results[c]["outx"].T.astype(np.float32)
        )
    return out.reshape(B, D, S)
